# revision 2
# baseline (speedup 1.0000x reference)
"""Trainium2 Bass kernel for nn_Attention_6133213298828.

Batch-parallel multi-head attention with per-query-position relative-position
logits, forward pass only. Data-parallel over 8 NeuronCores (batch dim);
weights replicated, no collectives.

Per-core design (2048 batches, tokens stored batch-major, 17 tokens/batch):
  - Work in fp16 (operand precision ~5e-4 rel err vs fp32 reference).
  - All matmul contractions need the contracted dim on SBUF partitions, so x
    is transposed on the PE (fp16 transpose-mode writes fp16 PSUM -> 2x-mode
    evictions).
  - Attention runs on 119-token groups (7 batches x 17 positions <= 128
    partitions). Scores for a whole group are one matmul per head producing
    dots^T; cross-batch garbage is suppressed by an additive -30 mask that is
    folded into the same matmul as 9 extra contraction rows of host-built
    indicator patterns. The relative-position logits are folded in the same
    way: 17 extra contraction rows pair a static one-hot position pattern
    with per-token rel projections (q @ KRABS[pos]) computed batched over
    the batch dim with host-premultiplied weights W_q @ KRABS.
  - Softmax: logits are bounded (|l| < ~4) so exp needs no max subtraction;
    denominators come from a ones column appended to V; attention stays
    unnormalized until the AV output is scaled by 1/denom per query row
    (per-partition scalar) during PSUM eviction.
  - AV output (token-major) is PE-transposed back to feature-major for the
    output projection.
"""

import numpy as np

DIM, OUT_DIM, H, V, B = 192, 192, 3, 17, 16384
DK = DIM // H
NCORES = 8
BC = B // NCORES          # batches per core
NB = 128                  # batches per chunk
NCHUNK = BC // NB         # 16
TC = NB * V               # 2176 tokens per chunk
TOK = BC * V              # 34816 tokens per core
GSIZES = [119] * 18 + [34]            # token-group sizes within a chunk
GOFFS = np.cumsum([0] + GSIZES).tolist()
G = len(GSIZES)           # 19
NGH = G * H               # 57 (group, head) tiles per chunk
MASKC = float(np.sqrt(30.0))
SCALE = DIM ** -0.5

_CACHED = {}


def _build_host_constants(W_qkv, b_qkv, key_rel, key_rel_diag, W_out, b_out):
    f16 = np.float16
    scale = np.float32(SCALE)

    # QK projection weights, q columns pre-scaled so Q^T comes out scaled.
    # Column order chosen so matmul operand pairs share a base partition:
    # slabA = [q^h0; q^h1], slabB = [k_h0; k_h1], slabC = [q^h2; k_h2].
    qs = W_qkv[:, 0:DIM] * scale
    kk = W_qkv[:, DIM:2 * DIM]
    wqk = np.concatenate(
        [qs[:, 0:128], kk[:, 0:128], qs[:, 128:192], kk[:, 128:192]], axis=1)
    wv = W_qkv[:, 2 * DIM:3 * DIM]

    # KRABS[i, j] = relative key vector seen by query position i at absolute
    # key position j (diag vector on j == i).
    kr = key_rel.reshape(V, V - 1, DK)
    KRABS = np.zeros((V, V, DK), np.float32)
    for i in range(V):
        for j in range(V):
            KRABS[i, j] = key_rel_diag[0] if j == i else kr[i, j - (j > i)]

    # wrel78[i]: (192, 78) fp16. Columns 26h + j' (j' < 17) hold
    # scale * W_q[:, head h] @ KRABS[i, j']; columns 26h+17..26h+25 are zero
    # (they are overwritten by the static mask rows of FRM after eviction).
    wrel = np.zeros((V, DIM, 96), np.float32)
    for h in range(H):
        wq_h = W_qkv[:, h * DK:(h + 1) * DK]          # (192, 64)
        proj = np.einsum('dk,ijk->dij', wq_h, KRABS) * scale   # (192, 17i, 17j)
        for i in range(V):
            wrel[i, :, 32 * h:32 * h + 17] = proj[:, i, :]

    # Static patterns over a chunk's 2176 tokens.
    t = np.arange(TC)
    pos = t % V               # position within sequence
    grp = (t // V) % 7        # batch index within 119-token group
    eml = np.zeros((26, TC), np.float32)
    for j in range(V):
        eml[j] = (pos == j)
    for a in range(7):
        eml[17 + a] = MASKC * (grp == a)
    eml[24] = 0.0
    eml[25] = 1.0
    maskr = np.zeros((9, TC), np.float32)
    for a in range(7):
        maskr[a] = MASKC * (grp == a)
    maskr[7] = 0.0
    maskr[8] = -30.0

    # eml rows 17..24 must pair with maskr rows 0..7: EML has 8 mask rows +
    # ones row -> rows 17..24 = maskL a=0..6 plus one unused, row 25 = ones.
    # maskr rows: 0..6 = a-patterns, 7 unused, 8 = -30. Fix alignment: EML
    # row 17+a pairs with FRM row 17+a. FRM rows 17..25 = maskr rows 0..8.
    # So eml rows 17..23 <- a=0..6, row 24 unused (0), row 25 ones pairs with
    # maskr row 8 = -30.
    emlp = np.concatenate([eml, np.zeros((6, TC), np.float32)], axis=0)  # pad 26->32
    eml3 = np.concatenate([emlp, emlp, emlp], axis=0)   # (96, TC), blocks at 32h

    consts = {
        "wqk0": wqk[0:128].astype(f16),
        "wqk1": wqk[128:192].astype(f16),
        "wv0": wv[0:128].astype(f16),
        "wv1": wv[128:192].astype(f16),
        "wout0": W_out[0:128].astype(f16),
        "wout1": W_out[128:192].astype(f16),
        "wrel0": wrel[:, 0:128, :].reshape(V * 128, 96).astype(f16),
        "wrel1": wrel[:, 128:192, :].reshape(V * 64, 96).astype(f16),
        "eml": eml3.astype(f16),
        "maskr": maskr.astype(f16),
        "ident": np.eye(128, dtype=f16),
    }
    return consts


def _build_bass():
    import concourse.bacc as bacc
    import concourse.mybir as mybir
    from concourse import tile

    f16 = mybir.dt.float16
    f32 = mybir.dt.float32
    EXP = mybir.ActivationFunctionType.Exp

    nc = bacc.Bacc(None, target_bir_lowering=False)

    x_in = nc.declare_dram_parameter("x", [TOK, DIM], f32, isOutput=False)
    dp = lambda name, shape: nc.declare_dram_parameter(name, list(shape), f16, isOutput=False)
    wqk0_d = dp("wqk0", (128, 384)); wqk1_d = dp("wqk1", (64, 384))
    wv0_d = dp("wv0", (128, 192)); wv1_d = dp("wv1", (64, 192))
    wout0_d = dp("wout0", (128, 192)); wout1_d = dp("wout1", (64, 192))
    wrel0_d = dp("wrel0", (V * 128, 96)); wrel1_d = dp("wrel1", (V * 64, 96))
    eml_d = dp("eml", (96, TC)); maskr_d = dp("maskr", (9, TC))
    ident_d = dp("ident", (128, 128))
    y_out = nc.declare_dram_parameter("y", [TOK, DIM], f32, isOutput=True)

    NT512 = [(0, 512), (512, 512), (1024, 512), (1536, 512), (2048, 128)]

    with tile.TileContext(nc) as tc:
        with tc.sbuf_pool(name="wpool", bufs=1) as wp, \
             tc.sbuf_pool(name="work", bufs=2) as sp, \
             tc.psum_pool(name="ps", bufs=3) as ps, \
             tc.psum_pool(name="pst", bufs=2) as pst:

            # ---- persistent weights ----
            wqk0 = wp.tile([128, 384], f16); nc.sync.dma_start(out=wqk0[:], in_=wqk0_d[:])
            wqk1 = wp.tile([64, 384], f16); nc.sync.dma_start(out=wqk1[:], in_=wqk1_d[:])
            wv0 = wp.tile([128, 192], f16); nc.sync.dma_start(out=wv0[:], in_=wv0_d[:])
            wv1 = wp.tile([64, 192], f16); nc.sync.dma_start(out=wv1[:], in_=wv1_d[:])
            wout0 = wp.tile([128, 192], f16); nc.sync.dma_start(out=wout0[:], in_=wout0_d[:])
            wout1 = wp.tile([64, 192], f16); nc.sync.dma_start(out=wout1[:], in_=wout1_d[:])
            wrel0 = wp.tile([128, V * 96], f16)
            nc.sync.dma_start(out=wrel0[:].rearrange("p (i j) -> p i j", j=96),
                              in_=wrel0_d[:].rearrange("(i p) j -> p i j", p=128))
            wrel1 = wp.tile([64, V * 96], f16)
            nc.sync.dma_start(out=wrel1[:].rearrange("p (i j) -> p i j", j=96),
                              in_=wrel1_d[:].rearrange("(i p) j -> p i j", p=64))
            eml = wp.tile([96, TC], f16); nc.sync.dma_start(out=eml[:], in_=eml_d[:])
            ident = wp.tile([128, 128], f16); nc.sync.dma_start(out=ident[:], in_=ident_d[:])

            for c in range(NCHUNK):
                r0 = c * TC
                # ---- load x chunk (cast f32 -> f16) ----
                xnat = sp.tile([128, V * DIM], f16, tag="xnat")
                nc.gpsimd.dma_start(
                    out=xnat[:].rearrange("p (t d) -> p t d", d=DIM),
                    in_=x_in[r0:r0 + TC, :].rearrange("(t p) d -> p t d", p=128))

                # ---- x^T via PE transpose ----
                xt0 = sp.tile([128, TC], f16, tag="xt0")
                xt1 = sp.tile([64, TC], f16, tag="xt1")
                for tp in range(5):          # packs of 4 token-tiles
                    n = min(4, 17 - tp * 4)
                    pa = pst.tile([128, 512], f16, tag="pst")
                    pb = pst.tile([64, 512], f16, tag="pstb")
                    for u in range(n):
                        t = tp * 4 + u
                        nc.tensor.transpose(pa[:, u * 128:(u + 1) * 128],
                                            xnat[:, t * DIM:t * DIM + 128], ident[:])
                        nc.tensor.transpose(pb[:, u * 128:(u + 1) * 128],
                                            xnat[:, t * DIM + 128:t * DIM + 192],
                                            ident[:])
                    cs = slice(tp * 512, tp * 512 + n * 128)
                    nc.vector.tensor_copy(xt0[:, cs], pa[:, 0:n * 128])
                    nc.scalar.copy(xt1[:, cs], pb[:, 0:n * 128])

                # ---- QK^T projections -> 3 slabs ----
                # slabA=[q^h0;q^h1]  slabB=[k_h0;k_h1]  slabC=[q^h2;k_h2]
                qka = sp.tile([128, TC], f16, tag="qka")
                qkb = sp.tile([128, TC], f16, tag="qkb")
                qkc = sp.tile([128, TC], f16, tag="qkc")
                slabs = [qka, qkb, qkc]
                for m in range(3):
                    for ni, (n0, nw) in enumerate(NT512):
                        pq = ps.tile([128, 512], f32, tag="ps32")
                        nc.tensor.matmul(pq[:, 0:nw], wqk0[:, m * 128:(m + 1) * 128],
                                         xt0[:, n0:n0 + nw], start=True, stop=False)
                        nc.tensor.matmul(pq[:, 0:nw], wqk1[:, m * 128:(m + 1) * 128],
                                         xt1[:, n0:n0 + nw], start=False, stop=True)
                        if (m * 5 + ni) % 2 == 0:
                            nc.vector.tensor_copy(slabs[m][:, n0:n0 + nw], pq[:, 0:nw])
                        else:
                            nc.scalar.copy(slabs[m][:, n0:n0 + nw], pq[:, 0:nw])

                # ---- rel projections -> frm (78, TC) ----
                # frm rows 26h+[0,17) = Frel_h ; rows 26h+[17,26) = mask rows
                frm = sp.tile([96, TC], f16, tag="frm")
                xt0v = xt0[:].rearrange("p (b v) -> p b v", v=V)
                xt1v = xt1[:].rearrange("p (b v) -> p b v", v=V)
                for ip in range(5):          # packs of 4 positions
                    n = min(4, V - ip * 4)
                    pr = ps.tile([96, 512], f32, tag="ps32")
                    for u in range(n):
                        i = ip * 4 + u
                        nc.tensor.matmul(pr[:, u * 128:u * 128 + 128],
                                         wrel0[:, i * 96:(i + 1) * 96],
                                         xt0v[:, :, i], start=True, stop=False)
                        nc.tensor.matmul(pr[:, u * 128:u * 128 + 128],
                                         wrel1[:, i * 96:(i + 1) * 96],
                                         xt1v[:, :, i], start=False, stop=True)
                    for u in range(n):
                        i = ip * 4 + u
                        nc.vector.tensor_copy(
                            frm[:].rearrange("p (b v) -> p b v", v=V)[:, :, i],
                            pr[:, u * 128:u * 128 + 128])
                # restore static mask rows (evict wrote zeros there)
                for h in range(H):
                    nc.sync.dma_start(out=frm[32 * h + 17:32 * h + 26, :], in_=maskr_d[:])

                # ---- dots^T + rel + mask, exp ----
                # k_h2 must sit at base partition 0 to pair with q^h2
                kh2t = sp.tile([64, TC], f16, tag="kh2t")
                nc.vector.tensor_copy(kh2t[:], qkc[64:128, :])
                QT = [qka[0:64, :], qka[64:128, :], qkc[0:64, :]]
                KT = [qkb[0:64, :], qkb[64:128, :], kh2t[0:64, :]]
                attn = sp.tile([119, NGH * 119], f16, tag="attn")
                for pk in range(15):         # packs of 4 (g,h) tiles; 57 = 14*4+1
                    n = min(4, NGH - pk * 4)
                    if n <= 0:
                        break
                    pd = ps.tile([119, 476], f32, tag="ps32")
                    for u in range(n):
                        idx = pk * 4 + u
                        g, h = divmod(idx, H)
                        gs = GSIZES[g]
                        gc = slice(GOFFS[g], GOFFS[g] + gs)
                        o = u * 119
                        nc.tensor.matmul(pd[0:gs, o:o + gs], KT[h][:, gc], QT[h][:, gc],
                                         start=True, stop=False)
                        nc.tensor.matmul(pd[0:gs, o:o + gs],
                                         eml[32 * h:32 * h + 26, gc],
                                         frm[32 * h:32 * h + 26, gc],
                                         start=False, stop=True)
                    nc.scalar.activation(attn[:, pk * 476:pk * 476 + n * 119],
                                         pd[:, 0:n * 119], EXP)

                # ---- V projection ----
                vt = sp.tile([119, G * 195], f16, tag="vt")
                nc.gpsimd.memset(
                    vt[:].rearrange("p (g hh c) -> p g hh c", hh=3, c=65)[:, :, :, 64:65],
                    1.0)
                for gp in range(10):         # packs of 2 groups; 19 = 9*2+1
                    n = min(2, G - gp * 2)
                    pv = ps.tile([119, 384], f32, tag="ps32")
                    for u in range(n):
                        g = gp * 2 + u
                        gs = GSIZES[g]
                        gc = slice(GOFFS[g], GOFFS[g] + gs)
                        nc.tensor.matmul(pv[0:gs, u * 192:u * 192 + 192],
                                         xt0[:, gc], wv0[:], start=True, stop=False)
                        nc.tensor.matmul(pv[0:gs, u * 192:u * 192 + 192],
                                         xt1[:, gc], wv1[:], start=False, stop=True)
                    g0 = gp * 2
                    src = pv[:].rearrange("p (g hh c) -> p g hh c", hh=3, c=64)[:, 0:n]
                    dst = vt[:].rearrange("p (g hh c) -> p g hh c", hh=3, c=65)[
                        :, g0:g0 + n, :, 0:64]
                    if gp % 2 == 0:
                        nc.vector.tensor_copy(dst, src)
                    else:
                        nc.scalar.copy(dst, src)

                # ---- attention @ V (+denominator), normalize ----
                avout = sp.tile([119, G * 192], f16, tag="avout")
                recip = sp.tile([119, NGH], f32, tag="recip")
                vtv = vt[:].rearrange("p (g c) -> p g c", c=195)
                for gp in range(10):         # packs of 2 groups
                    n = min(2, G - gp * 2)
                    pa = ps.tile([119, 390], f32, tag="ps32")
                    for u in range(n):
                        g = gp * 2 + u
                        gs = GSIZES[g]
                        for h in range(H):
                            idx = g * H + h
                            nc.tensor.matmul(
                                pa[0:gs, u * 195 + 65 * h:u * 195 + 65 * h + 65],
                                attn[0:gs, idx * 119:idx * 119 + gs],
                                vtv[0:gs, g, 65 * h:65 * h + 65],
                                start=True, stop=True)
                    g0 = gp * 2
                    nc.vector.reciprocal(
                        recip[:, g0 * H:(g0 + n) * H].rearrange("p (g hh) -> p g hh", hh=3),
                        pa[:].rearrange("p (g hh c) -> p g hh c", hh=3, c=65)[
                            :, 0:n, :, 64])
                    for u in range(n):
                        g = g0 + u
                        gs = GSIZES[g]
                        for h in range(H):
                            idx = g * H + h
                            src = pa[0:gs, u * 195 + 65 * h:u * 195 + 65 * h + 64]
                            dst = avout[0:gs, g * 192 + 64 * h:g * 192 + 64 * h + 64]
                            sc = recip[0:gs, idx:idx + 1]
                            if idx % 2 == 0:
                                nc.vector.tensor_scalar_mul(dst, src, sc)
                            else:
                                nc.scalar.activation(dst, src,
                                                     mybir.ActivationFunctionType.Copy,
                                                     scale=sc)

                # ---- transpose attnout back to feature-major ----
                aot0 = sp.tile([128, TC], f16, tag="aot0")
                aot1 = sp.tile([64, TC], f16, tag="aot1")
                for gp in range(5):          # packs of 4 groups
                    n = min(4, G - gp * 4)
                    # slot stride 120 keeps fp16 PSUM writes 4-byte aligned
                    pa = pst.tile([128, 480], f16, tag="pst")
                    pb = pst.tile([64, 480], f16, tag="pstb")
                    for u in range(n):
                        g = gp * 4 + u
                        gs = GSIZES[g]
                        nc.tensor.transpose(pa[:, u * 120:u * 120 + gs],
                                            avout[0:gs, g * 192:g * 192 + 128],
                                            ident[0:gs, 0:gs])
                        nc.tensor.transpose(pb[:, u * 120:u * 120 + gs],
                                            avout[0:gs, g * 192 + 128:g * 192 + 192],
                                            ident[0:gs, 0:gs])
                    t0 = GOFFS[gp * 4]
                    if n == 4 and GSIZES[gp * 4 + 3] == 119:
                        # uniform pack: one strided copy per slab
                        sa = pa[:].rearrange("p (u c) -> p u c", c=120)[:, :, 0:119]
                        sb = pb[:].rearrange("p (u c) -> p u c", c=120)[:, :, 0:119]
                        da = aot0[:, t0:t0 + 476].rearrange("p (u c) -> p u c", c=119)
                        db = aot1[:, t0:t0 + 476].rearrange("p (u c) -> p u c", c=119)
                        nc.vector.tensor_copy(da, sa)
                        nc.scalar.copy(db, sb)
                    else:
                        for u in range(n):
                            g = gp * 4 + u
                            gs = GSIZES[g]
                            gt = GOFFS[g]
                            if u % 2 == 0:
                                nc.vector.tensor_copy(aot0[:, gt:gt + gs],
                                                      pa[:, u * 120:u * 120 + gs])
                                nc.scalar.copy(aot1[:, gt:gt + gs],
                                               pb[:, u * 120:u * 120 + gs])
                            else:
                                nc.scalar.copy(aot0[:, gt:gt + gs],
                                               pa[:, u * 120:u * 120 + gs])
                                nc.vector.tensor_copy(aot1[:, gt:gt + gs],
                                                      pb[:, u * 120:u * 120 + gs])

                # ---- output projection ----
                fin = sp.tile([119, G * 192], f32, tag="fin")
                for gp in range(10):
                    n = min(2, G - gp * 2)
                    po = ps.tile([119, 384], f32, tag="ps32")
                    for u in range(n):
                        g = gp * 2 + u
                        gs = GSIZES[g]
                        gc = slice(GOFFS[g], GOFFS[g] + gs)
                        nc.tensor.matmul(po[0:gs, u * 192:u * 192 + 192],
                                         aot0[:, gc], wout0[:], start=True, stop=False)
                        nc.tensor.matmul(po[0:gs, u * 192:u * 192 + 192],
                                         aot1[:, gc], wout1[:], start=False, stop=True)
                    g0 = gp * 2
                    dst = fin[:, g0 * 192:(g0 + n) * 192]
                    if gp % 2 == 0:
                        nc.vector.tensor_copy(dst, po[:, 0:n * 192])
                    else:
                        nc.scalar.copy(dst, po[:, 0:n * 192])

                # ---- store ----
                nc.sync.dma_start(
                    out=y_out[r0:r0 + 18 * 119, :].rearrange("(g p) d -> p g d", p=119),
                    in_=fin[:].rearrange("p (g d) -> p g d", d=192)[:, 0:18, :])
                nc.sync.dma_start(
                    out=y_out[r0 + 18 * 119:r0 + TC, :],
                    in_=fin[0:34, 18 * 192:19 * 192])

    nc.finalize()
    return nc


def kernel(x, W_qkv, b_qkv, key_rel, key_rel_diag, W_out, b_out):
    from concourse.bass_utils import run_bass_kernel_spmd

    x = np.ascontiguousarray(np.asarray(x, dtype=np.float32))
    consts = _build_host_constants(
        np.asarray(W_qkv, np.float32), np.asarray(b_qkv, np.float32),
        np.asarray(key_rel, np.float32), np.asarray(key_rel_diag, np.float32),
        np.asarray(W_out, np.float32), np.asarray(b_out, np.float32))

    if "nc" not in _CACHED:
        _CACHED["nc"] = _build_bass()
    nc = _CACHED["nc"]

    xs = x.reshape(NCORES, BC * V, DIM)
    in_maps = [dict(consts, x=xs[k]) for k in range(NCORES)]
    res = run_bass_kernel_spmd(nc, in_maps, core_ids=list(range(NCORES)))
    _CACHED["last_result"] = res
    out = np.stack([res.results[k]["y"] for k in range(NCORES)], axis=0)
    return out.reshape(B, V, DIM)



# revision 12
# speedup vs baseline: 1.3312x; 1.3312x over previous
"""Trainium2 Bass kernel for nn_Attention_6133213298828 (v2).

Batch-parallel multi-head attention with relative-position logits, forward
only. Data-parallel over 8 NeuronCores (batch dim); weights replicated.

v2 design (per core: 2048 batches = 16 chunks x 128 batches x 17 tokens):
  - fp16 I/O: x is pre-cast to fp16 on host, y is produced fp16 and upcast
    on host -> halves HBM traffic.
  - x^T comes straight from HBM via two xbar DMA-transposes per chunk
    (no on-chip transposes of x, no xnat tile).
  - Attention runs on OVERLAPPED 128-token key/query groups with stride
    119 (= 7 batches x 17). Groups own 119 "real" queries; the 9 padded
    queries/keys belong to the following batch and are masked / discarded.
    This makes every hot matmul stationary exactly 128 columns (FWL).
  - Scores per (g,h): TWO accumulating matmuls into one fp32 psum tile
    [128k x 128q]: (1) mask+rel matmul: static stationary eml90 pattern
    block (26 rows at partition 32h) x frm moving (rel projections +
    static query-mask rows), then (2) K^T x Q (64 rows). Shallow matmul
    first so the deeper one's per-element drain always lands later.
  - Softmax: bounded logits -> exp without max-subtraction; denominator
    from a ones column in V; normalization folded into one
    tensor_tensor multiply per AV psum tile with a broadcast reciprocal.
  - AV, V-proj, out-proj stationaries all 128 columns (overlapped groups).
"""

import numpy as np

DIM, OUT_DIM, H, V, B = 192, 192, 3, 17, 16384
DK = DIM // H
NCORES = 8
BC = B // NCORES          # batches per core
NB = 128                  # batches per chunk
NCHUNK = BC // NB         # 16
TC = NB * V               # 2176 tokens per chunk
TOK = BC * V              # 34816 tokens per core
GS = 119                  # group stride (7 batches x 17)
G = 19                    # groups per chunk (18 full-stride + tail)
MASKC = float(np.sqrt(30.0))
SCALE = DIM ** -0.5

def _gdims(g):
    """(token_start, n_keys, n_queries) for group g within a chunk."""
    t0 = g * GS
    n = min(128, TC - t0)
    return t0, n, n

_CACHED = {}


def _build_host_constants(W_qkv, b_qkv, key_rel, key_rel_diag, W_out, b_out):
    f16 = np.float16
    scale = np.float32(SCALE)

    # Per-head slabs: slab h = [q_h * scale (64) | k_h (64)] except the
    # baseline-compatible pairing: A=[q0;q1] B=[k0;k1] C=[q2;k2].
    qs = W_qkv[:, 0:DIM] * scale
    kk = W_qkv[:, DIM:2 * DIM]
    wqk = np.concatenate(
        [qs[:, 0:128], kk[:, 0:128], qs[:, 128:192], kk[:, 128:192]], axis=1)
    wv = W_qkv[:, 2 * DIM:3 * DIM]

    # KRABS[i, j] = relative key vector seen by query position i at key
    # position j (diag on j == i).
    kr = key_rel.reshape(V, V - 1, DK)
    KRABS = np.zeros((V, V, DK), np.float32)
    for i in range(V):
        for j in range(V):
            KRABS[i, j] = key_rel_diag[0] if j == i else kr[i, j - (j > i)]

    # wrel[i]: (192, 96); cols 32h + j (j < 17) = scale * W_qh @ KRABS[i, j]
    wrel = np.zeros((V, DIM, 96), np.float32)
    for h in range(H):
        wq_h = W_qkv[:, h * DK:(h + 1) * DK]
        proj = np.einsum('dk,ijk->dij', wq_h, KRABS) * scale
        for i in range(V):
            wrel[i, :, 32 * h:32 * h + 17] = proj[:, i, :]

    # eml90: static key-side mask/position pattern over group-local l=0..127,
    # replicated at partition offsets 0/32/64 (one block per head).
    # rows 0-16: one-hot(l mod 17 == r); rows 17-24: MASKC*(l//17 == a),
    # a = 0..7; row 25: ones.
    l = np.arange(128)
    eml1 = np.zeros((32, 128), np.float32)
    for r in range(V):
        eml1[r] = (l % V == r)
    for a in range(8):
        eml1[17 + a] = MASKC * (l // V == a)
    eml1[25] = 1.0
    eml90 = np.concatenate([eml1, eml1, eml1], axis=0)[0:90]

    # maskq: static query-side rows of frm (rows 17-25 of each head block):
    # rows 0-6: MASKC*((t mod 119)//17 == b); row 7: 0; row 8: -30.
    t = np.arange(TC)
    bq = (t % GS) // V
    maskq = np.zeros((9, TC), np.float32)
    for b in range(7):
        maskq[b] = MASKC * (bq == b)
    maskq[8] = -30.0

    consts = {
        "wqk0": wqk[0:128].astype(f16),
        "wqk1": wqk[128:192].astype(f16),
        "wv0": wv[0:128].astype(f16),
        "wv1": wv[128:192].astype(f16),
        "wout0": W_out[0:128].astype(f16),
        "wout1": W_out[128:192].astype(f16),
        "wrel0": wrel[:, 0:128, :].reshape(V * 128, 96).astype(f16),
        "wrel1": wrel[:, 128:192, :].reshape(V * 64, 96).astype(f16),
        "eml90": eml90.astype(f16),
        "maskq": maskq.astype(f16),
        "ident": np.eye(128, dtype=f16),
    }
    return consts


def _build_bass():
    import concourse.bacc as bacc
    import concourse.mybir as mybir
    from concourse import tile

    f16 = mybir.dt.float16
    f32 = mybir.dt.float32
    EXP = mybir.ActivationFunctionType.Exp

    nc = bacc.Bacc(None, target_bir_lowering=False)

    # x arrives pre-transposed per chunk: [NCHUNK * 192, TC], chunk c rows
    # c*192 .. c*192+192 hold x[chunk c]^T (feature-major).
    x_in = nc.declare_dram_parameter("x", [NCHUNK * DIM, TC], f16, isOutput=False)
    dp = lambda name, shape: nc.declare_dram_parameter(name, list(shape), f16, isOutput=False)
    wqk0_d = dp("wqk0", (128, 384)); wqk1_d = dp("wqk1", (64, 384))
    wv0_d = dp("wv0", (128, 192)); wv1_d = dp("wv1", (64, 192))
    wout0_d = dp("wout0", (128, 192)); wout1_d = dp("wout1", (64, 192))
    wrel0_d = dp("wrel0", (V * 128, 96)); wrel1_d = dp("wrel1", (V * 64, 96))
    eml90_d = dp("eml90", (90, 128)); maskq_d = dp("maskq", (9, TC))
    ident_d = dp("ident", (128, 128))
    y_out = nc.declare_dram_parameter("y", [TOK, DIM], f16, isOutput=True)

    NT512 = [(0, 512), (512, 512), (1024, 512), (1536, 512), (2048, 128)]

    with tile.TileContext(nc) as tc:
        with tc.sbuf_pool(name="wpool", bufs=1) as wp, \
             tc.sbuf_pool(name="work", bufs=2) as sp, \
             tc.psum_pool(name="ps", bufs=3) as ps, \
             tc.psum_pool(name="pst", bufs=2) as pst:

            # ---- persistent weights ----
            wqk0 = wp.tile([128, 384], f16); nc.sync.dma_start(out=wqk0[:], in_=wqk0_d[:])
            wqk1 = wp.tile([64, 384], f16); nc.sync.dma_start(out=wqk1[:], in_=wqk1_d[:])
            wv0 = wp.tile([128, 192], f16); nc.sync.dma_start(out=wv0[:], in_=wv0_d[:])
            wv1 = wp.tile([64, 192], f16); nc.sync.dma_start(out=wv1[:], in_=wv1_d[:])
            wout0 = wp.tile([128, 192], f16); nc.sync.dma_start(out=wout0[:], in_=wout0_d[:])
            wout1 = wp.tile([64, 192], f16); nc.sync.dma_start(out=wout1[:], in_=wout1_d[:])
            wrel0 = wp.tile([128, V * 96], f16)
            nc.sync.dma_start(out=wrel0[:].rearrange("p (i j) -> p i j", j=96),
                              in_=wrel0_d[:].rearrange("(i p) j -> p i j", p=128))
            wrel1 = wp.tile([64, V * 96], f16)
            nc.sync.dma_start(out=wrel1[:].rearrange("p (i j) -> p i j", j=96),
                              in_=wrel1_d[:].rearrange("(i p) j -> p i j", p=64))
            eml90 = wp.tile([90, 128], f16); nc.sync.dma_start(out=eml90[:], in_=eml90_d[:])
            ident = wp.tile([128, 128], f16); nc.sync.dma_start(out=ident[:], in_=ident_d[:])

            for c in range(NCHUNK):
                r0 = c * TC
                # ---- x^T loaded directly (host pre-transposed) ----
                xt0 = sp.tile([128, TC], f16, tag="xt0")
                xt1t = sp.tile([64, TC], f16, tag="xt1")
                nc.sync.dma_start(out=xt0[:], in_=x_in[c * DIM:c * DIM + 128, :])
                nc.scalar.dma_start(out=xt1t[:], in_=x_in[c * DIM + 128:(c + 1) * DIM, :])
                xt1 = xt1t[0:64]

                # ---- QK projections -> 3 slabs ----
                # slabA=[q0;q1] slabB=[k0;k1] slabC=[q2;k2]
                qka = sp.tile([128, TC], f16, tag="qka")
                qkb = sp.tile([128, TC], f16, tag="qkb")
                qkc = sp.tile([128, TC], f16, tag="qkc")
                slabs = [qka, qkb, qkc]
                for m in range(3):
                    for ni, (n0, nw) in enumerate(NT512):
                        pq = ps.tile([128, 512], f32, tag="ps32")
                        nc.tensor.matmul(pq[:, 0:nw], wqk0[:, m * 128:(m + 1) * 128],
                                         xt0[:, n0:n0 + nw], start=True, stop=False)
                        nc.tensor.matmul(pq[:, 0:nw], wqk1[:, m * 128:(m + 1) * 128],
                                         xt1[:, n0:n0 + nw], start=False, stop=True)
                        if (m * 5 + ni) % 2 == 0:
                            nc.vector.tensor_copy(slabs[m][:, n0:n0 + nw], pq[:, 0:nw])
                        else:
                            nc.scalar.copy(slabs[m][:, n0:n0 + nw], pq[:, 0:nw])

                # ---- rel projections -> frm (96, TC) ----
                # frm rows 32h+[0,17) = rel logits; rows 32h+[17,26) static
                # query-mask rows (DMA'd once per pool buffer at c<2).
                frm = sp.tile([96, TC], f16, tag="frm")
                for h in range(H):
                    nc.sync.dma_start(out=frm[32 * h + 17:32 * h + 26, :],
                                      in_=maskq_d[:])
                xt0v = xt0[:].rearrange("p (b v) -> p b v", v=V)
                xt1v = xt1.rearrange("p (b v) -> p b v", v=V)
                frmv = frm[:].rearrange("p (b v) -> p b v", v=V)
                for ip in range(5):          # packs of 4 positions
                    n = min(4, V - ip * 4)
                    pr = ps.tile([96, 512], f32, tag="ps32")
                    for u in range(n):
                        i = ip * 4 + u
                        nc.tensor.matmul(pr[:, u * 128:u * 128 + 128],
                                         wrel0[:, i * 96:(i + 1) * 96],
                                         xt0v[:, :, i], start=True, stop=False)
                        nc.tensor.matmul(pr[:, u * 128:u * 128 + 128],
                                         wrel1[:, i * 96:(i + 1) * 96],
                                         xt1v[:, :, i], start=False, stop=True)
                    # evict rows 32h..32h+16 per head, all n positions at once
                    prv = pr[:].rearrange("p (u b) -> p b u", b=128)
                    for h in range(H):
                        src = prv[32 * h:32 * h + 17, :, 0:n]
                        dst = frmv[32 * h:32 * h + 17, :, ip * 4:ip * 4 + n]
                        if (ip + h) % 2 == 0:
                            nc.vector.tensor_copy(dst, src)
                        else:
                            nc.scalar.copy(dst, src)

                # ---- k_h2 to base partition 0 ----
                kh2t = sp.tile([64, TC], f16, tag="kh2t")
                nc.vector.tensor_copy(kh2t[:], qkc[64:128, :])
                QT = [qka[0:64, :], qka[64:128, :], qkc[0:64, :]]
                KT = [qkb[0:64, :], qkb[64:128, :], kh2t[0:64, :]]

                # ---- scores: mask+rel matmul then K^T Q, exp ----
                attn = sp.tile([128, 57 * 128], f16, tag="attn")
                for pk in range(15):         # packs of 4 (g,h) tiles; 57=14*4+1
                    n = min(4, 57 - pk * 4)
                    pd = ps.tile([128, 512], f32, tag="ps32")
                    for u in range(n):
                        idx = pk * 4 + u
                        g, h = divmod(idx, H)
                        t0, kn, qn = _gdims(g)
                        gk = slice(t0, t0 + kn)
                        gq = slice(t0, t0 + qn)
                        o = u * 128
                        nc.tensor.matmul(pd[0:kn, o:o + qn],
                                         eml90[32 * h:32 * h + 26, 0:kn],
                                         frm[32 * h:32 * h + 26, gq],
                                         start=True, stop=False)
                        nc.tensor.matmul(pd[0:kn, o:o + qn], KT[h][:, gk],
                                         QT[h][:, gq], start=False, stop=True)
                    nc.scalar.activation(attn[:, pk * 512:pk * 512 + n * 128],
                                         pd[:, 0:n * 128], EXP)

                # ---- V projection (overlapped 128-token groups) ----
                vt = sp.tile([128, G * 195], f16, tag="vt")
                nc.gpsimd.memset(
                    vt[:].rearrange("p (g hh c) -> p g hh c", hh=3, c=65)[:, :, :, 64:65],
                    1.0)
                for gp in range(10):         # packs of 2 groups
                    n = min(2, G - gp * 2)
                    pv = ps.tile([128, 384], f32, tag="ps32")
                    for u in range(n):
                        g = gp * 2 + u
                        t0, kn, qn = _gdims(g)
                        gk = slice(t0, t0 + kn)
                        nc.tensor.matmul(pv[0:kn, u * 192:u * 192 + 192],
                                         xt0[:, gk], wv0[:], start=True, stop=False)
                        nc.tensor.matmul(pv[0:kn, u * 192:u * 192 + 192],
                                         xt1[:, gk], wv1[:], start=False, stop=True)
                    g0 = gp * 2
                    src = pv[:].rearrange("p (g hh c) -> p g hh c", hh=3, c=64)[:, 0:n]
                    dst = vt[:].rearrange("p (g hh c) -> p g hh c", hh=3, c=65)[
                        :, g0:g0 + n, :, 0:64]
                    if gp % 2 == 0:
                        nc.vector.tensor_copy(dst, src)
                    else:
                        nc.scalar.copy(dst, src)

                # ---- attention @ V (+denominator), normalize via TT ----
                avout = sp.tile([128, G * 192], f16, tag="avout")
                recip = sp.tile([128, G * H], f32, tag="recip")
                vtv = vt[:].rearrange("p (g c) -> p g c", c=195)
                for gp in range(10):         # packs of 2 groups
                    n = min(2, G - gp * 2)
                    pa = ps.tile([128, 390], f32, tag="ps32")
                    for u in range(n):
                        g = gp * 2 + u
                        t0, kn, qn = _gdims(g)
                        for h in range(H):
                            idx = g * H + h
                            nc.tensor.matmul(
                                pa[0:qn, u * 195 + 65 * h:u * 195 + 65 * h + 65],
                                attn[0:kn, idx * 128:idx * 128 + qn],
                                vtv[0:kn, g, 65 * h:65 * h + 65],
                                start=True, stop=True)
                    g0 = gp * 2
                    pav = pa[:].rearrange("p (g hh c) -> p g hh c", hh=3, c=65)
                    nc.vector.reciprocal(
                        recip[:, g0 * H:(g0 + n) * H].rearrange(
                            "p (g hh) -> p g hh", hh=3),
                        pav[:, 0:n, :, 64])
                    rb = recip[:, g0 * H:(g0 + n) * H].rearrange(
                        "p (g hh) -> p g hh", hh=3).unsqueeze(3).broadcast_to(
                        (128, n, 3, 64))
                    nc.vector.tensor_mul(
                        avout[:].rearrange("p (g hh c) -> p g hh c", hh=3, c=64)[
                            :, g0:g0 + n],
                        pav[:, 0:n, :, 0:64], rb)

                # ---- transpose avout to feature-major ----
                aot0 = sp.tile([128, G * 128], f16, tag="aot0")
                aot1 = sp.tile([64, G * 128], f16, tag="aot1")
                for gp in range(5):          # packs of 4 groups
                    n = min(4, G - gp * 4)
                    pta = pst.tile([128, 512], f16, tag="pst")
                    ptb = pst.tile([64, 512], f16, tag="pstb")
                    for u in range(n):
                        g = gp * 4 + u
                        t0, kn, qn = _gdims(g)
                        nc.tensor.transpose(pta[:, u * 128:u * 128 + qn],
                                            avout[0:qn, g * 192:g * 192 + 128],
                                            ident[0:qn, 0:qn])
                        nc.tensor.transpose(ptb[:, u * 128:u * 128 + qn],
                                            avout[0:qn, g * 192 + 128:g * 192 + 192],
                                            ident[0:qn, 0:qn])
                    cs = slice(gp * 512, gp * 512 + n * 128)
                    nc.vector.tensor_copy(aot0[:, cs], pta[:, 0:n * 128])
                    nc.scalar.copy(aot1[:, cs], ptb[:, 0:n * 128])

                # ---- output projection ----
                fin = sp.tile([119, G * 192], f16, tag="fin")
                for gp in range(10):
                    n = min(2, G - gp * 2)
                    po = ps.tile([128, 384], f32, tag="ps32")
                    for u in range(n):
                        g = gp * 2 + u
                        t0, kn, qn = _gdims(g)
                        nc.tensor.matmul(po[0:qn, u * 192:u * 192 + 192],
                                         aot0[:, g * 128:g * 128 + qn],
                                         wout0[:], start=True, stop=False)
                        nc.tensor.matmul(po[0:qn, u * 192:u * 192 + 192],
                                         aot1[:, g * 128:g * 128 + qn],
                                         wout1[:], start=False, stop=True)
                    g0 = gp * 2
                    dst = fin[:, g0 * 192:(g0 + n) * 192]
                    if gp % 2 == 0:
                        nc.scalar.copy(dst, po[0:119, 0:n * 192])
                    else:
                        nc.vector.tensor_copy(dst, po[0:119, 0:n * 192])

                # ---- store ----
                nc.sync.dma_start(
                    out=y_out[r0:r0 + 18 * GS, :].rearrange("(g p) d -> p g d", p=GS),
                    in_=fin[:].rearrange("p (g d) -> p g d", d=192)[:, 0:18, :])
                nc.sync.dma_start(
                    out=y_out[r0 + 18 * GS:r0 + TC, :],
                    in_=fin[0:34, 18 * 192:19 * 192])

    nc.finalize()
    return nc


def kernel(x, W_qkv, b_qkv, key_rel, key_rel_diag, W_out, b_out):
    from concourse.bass_utils import run_bass_kernel_spmd

    # pre-transpose per chunk on host: (core, chunk, TC, DIM) -> (core, chunk, DIM, TC)
    xt = np.ascontiguousarray(
        np.asarray(x, dtype=np.float16).reshape(NCORES, NCHUNK, TC, DIM)
        .transpose(0, 1, 3, 2)).reshape(NCORES, NCHUNK * DIM, TC)
    consts = _build_host_constants(
        np.asarray(W_qkv, np.float32), np.asarray(b_qkv, np.float32),
        np.asarray(key_rel, np.float32), np.asarray(key_rel_diag, np.float32),
        np.asarray(W_out, np.float32), np.asarray(b_out, np.float32))

    if "nc" not in _CACHED:
        _CACHED["nc"] = _build_bass()
    nc = _CACHED["nc"]

    in_maps = [dict(consts, x=xt[k]) for k in range(NCORES)]
    res = run_bass_kernel_spmd(nc, in_maps, core_ids=list(range(NCORES)))
    _CACHED["last_result"] = res
    out = np.stack([res.results[k]["y"] for k in range(NCORES)], axis=0)
    return out.reshape(B, V, DIM).astype(np.float32)


# revision 23
# speedup vs baseline: 1.3444x; 1.0099x over previous
"""Trainium2 Bass kernel for nn_Attention_6133213298828 (v2).

Batch-parallel multi-head attention with relative-position logits, forward
only. Data-parallel over 8 NeuronCores (batch dim); weights replicated.

v2 design (per core: 2048 batches = 16 chunks x 128 batches x 17 tokens):
  - fp16 I/O: x is pre-cast to fp16 on host, y is produced fp16 and upcast
    on host -> halves HBM traffic.
  - x^T comes straight from HBM via two xbar DMA-transposes per chunk
    (no on-chip transposes of x, no xnat tile).
  - Attention runs on OVERLAPPED 128-token key/query groups with stride
    119 (= 7 batches x 17). Groups own 119 "real" queries; the 9 padded
    queries/keys belong to the following batch and are masked / discarded.
    This makes every hot matmul stationary exactly 128 columns (FWL).
  - Scores per (g,h): TWO accumulating matmuls into one fp32 psum tile
    [128k x 128q]: (1) mask+rel matmul: static stationary eml90 pattern
    block (26 rows at partition 32h) x frm moving (rel projections +
    static query-mask rows), then (2) K^T x Q (64 rows). Shallow matmul
    first so the deeper one's per-element drain always lands later.
  - Softmax: bounded logits -> exp without max-subtraction; denominator
    from a ones column in V; normalization folded into one
    tensor_tensor multiply per AV psum tile with a broadcast reciprocal.
  - AV, V-proj, out-proj stationaries all 128 columns (overlapped groups).
"""

import numpy as np

DIM, OUT_DIM, H, V, B = 192, 192, 3, 17, 16384
DK = DIM // H
NCORES = 8
BC = B // NCORES          # batches per core
NB = 128                  # batches per chunk
NCHUNK = BC // NB         # 16
TC = NB * V               # 2176 tokens per chunk
TOK = BC * V              # 34816 tokens per core
GS = 119                  # group stride (7 batches x 17)
G = 19                    # groups per chunk (18 full-stride + tail)
MASKC = float(np.sqrt(30.0))
SCALE = DIM ** -0.5

def _gdims(g):
    """(token_start, n_keys, n_queries) for group g within a chunk."""
    t0 = g * GS
    n = min(GS, TC - t0)
    return t0, n, n

_CACHED = {}


def _build_host_constants(W_qkv, b_qkv, key_rel, key_rel_diag, W_out, b_out):
    f16 = np.float16
    scale = np.float32(SCALE)

    # Per-head slabs: slab h = [q_h * scale (64) | k_h (64)].
    qs = W_qkv[:, 0:DIM] * scale
    kk = W_qkv[:, DIM:2 * DIM]
    wqk = np.concatenate(
        [np.concatenate([qs[:, h * DK:(h + 1) * DK],
                         kk[:, h * DK:(h + 1) * DK]], axis=1)
         for h in range(H)], axis=1)
    wv = W_qkv[:, 2 * DIM:3 * DIM]

    # KRABS[i, j] = relative key vector seen by query position i at key
    # position j (diag on j == i).
    kr = key_rel.reshape(V, V - 1, DK)
    KRABS = np.zeros((V, V, DK), np.float32)
    for i in range(V):
        for j in range(V):
            KRABS[i, j] = key_rel_diag[0] if j == i else kr[i, j - (j > i)]

    # wrel[i]: (192, 96); cols 32h + j (j < 17) = scale * W_qh @ KRABS[i, j]
    wrel = np.zeros((V, DIM, 96), np.float32)
    for h in range(H):
        wq_h = W_qkv[:, h * DK:(h + 1) * DK]
        proj = np.einsum('dk,ijk->dij', wq_h, KRABS) * scale
        for i in range(V):
            wrel[i, :, 32 * h:32 * h + 17] = proj[:, i, :]

    # Static key-side rows of kf (kf rows 64..88): per TOKEN t, viewed as
    # group-local l = t mod 119 (also serves the overlap use l+119 of the
    # previous group -- consistent because 119 = 7*17 and overlap keys are
    # masked purely by the -30 ones-row, there is no a=7 indicator).
    # rows 0-16: one-hot(t mod 17 == r); 17-23: MASKC*((t mod 119)//17 == a);
    # row 24: ones.
    t = np.arange(TC)
    bq = (t % GS) // V
    emlk = np.zeros((25, TC), np.float32)
    for r in range(V):
        emlk[r] = (t % V == r)
    for a in range(7):
        emlk[17 + a] = MASKC * (bq == a)
    emlk[24] = 1.0

    # Static query-side rows of qf (qf rows 81..88):
    # rows 0-6: MASKC*((t mod 119)//17 == b); row 7: -30.
    maskq = np.zeros((8, TC), np.float32)
    for b in range(7):
        maskq[b] = MASKC * (bq == b)
    maskq[7] = -30.0

    consts = {
        "wqk0": wqk[0:128].astype(f16),
        "wqk1": wqk[128:192].astype(f16),
        "wv0": wv[0:128].astype(f16),
        "wv1": wv[128:192].astype(f16),
        "wout0": W_out[0:128].astype(f16),
        "wout1": W_out[128:192].astype(f16),
        "wrel0": wrel[:, 0:128, :].reshape(V * 128, 96).astype(f16),
        "wrel1": wrel[:, 128:192, :].reshape(V * 64, 96).astype(f16),
        "emlk": emlk.astype(f16),
        "maskq": maskq.astype(f16),
        "ident": np.eye(128, dtype=f16),
    }
    return consts


def _build_bass():
    import concourse.bacc as bacc
    import concourse.mybir as mybir
    from concourse import tile

    f16 = mybir.dt.float16
    f32 = mybir.dt.float32
    EXP = mybir.ActivationFunctionType.Exp

    nc = bacc.Bacc(None, target_bir_lowering=False)

    # x arrives pre-transposed per chunk: [NCHUNK * 192, TC], chunk c rows
    # c*192 .. c*192+192 hold x[chunk c]^T (feature-major).
    x_in = nc.declare_dram_parameter("x", [NCHUNK * DIM, TC], f16, isOutput=False)
    dp = lambda name, shape: nc.declare_dram_parameter(name, list(shape), f16, isOutput=False)
    wqk0_d = dp("wqk0", (128, 384)); wqk1_d = dp("wqk1", (64, 384))
    wv0_d = dp("wv0", (128, 192)); wv1_d = dp("wv1", (64, 192))
    wout0_d = dp("wout0", (128, 192)); wout1_d = dp("wout1", (64, 192))
    wrel0_d = dp("wrel0", (V * 128, 96)); wrel1_d = dp("wrel1", (V * 64, 96))
    emlk_d = dp("emlk", (25, TC)); maskq_d = dp("maskq", (8, TC))
    ident_d = dp("ident", (128, 128))
    y_out = nc.declare_dram_parameter("y", [TOK, DIM], f16, isOutput=True)

    NT512 = [(0, 512), (512, 512), (1024, 512), (1536, 512), (2048, 128)]

    with tile.TileContext(nc) as tc:
        with tc.sbuf_pool(name="wpool", bufs=1) as wp, \
             tc.sbuf_pool(name="work", bufs=2) as sp, \
             tc.psum_pool(name="ps", bufs=3) as ps, \
             tc.psum_pool(name="pst", bufs=2) as pst:

            # ---- persistent weights ----
            wqk0 = wp.tile([128, 384], f16); nc.sync.dma_start(out=wqk0[:], in_=wqk0_d[:])
            wqk1 = wp.tile([64, 384], f16); nc.sync.dma_start(out=wqk1[:], in_=wqk1_d[:])
            wv0 = wp.tile([128, 192], f16); nc.sync.dma_start(out=wv0[:], in_=wv0_d[:])
            wv1 = wp.tile([64, 192], f16); nc.sync.dma_start(out=wv1[:], in_=wv1_d[:])
            wout0 = wp.tile([128, 192], f16); nc.sync.dma_start(out=wout0[:], in_=wout0_d[:])
            wout1 = wp.tile([64, 192], f16); nc.sync.dma_start(out=wout1[:], in_=wout1_d[:])
            wrel0 = wp.tile([128, V * 96], f16)
            nc.sync.dma_start(out=wrel0[:].rearrange("p (i j) -> p i j", j=96),
                              in_=wrel0_d[:].rearrange("(i p) j -> p i j", p=128))
            wrel1 = wp.tile([64, V * 96], f16)
            nc.sync.dma_start(out=wrel1[:].rearrange("p (i j) -> p i j", j=96),
                              in_=wrel1_d[:].rearrange("(i p) j -> p i j", p=64))
            ident = wp.tile([128, 128], f16); nc.sync.dma_start(out=ident[:], in_=ident_d[:])

            for c in range(NCHUNK):
                r0 = c * TC
                # ---- x^T loaded directly (host pre-transposed) ----
                xt0 = sp.tile([128, TC], f16, tag="xt0")
                xt1t = sp.tile([64, TC], f16, tag="xt1")
                nc.sync.dma_start(out=xt0[:], in_=x_in[c * DIM:c * DIM + 128, :])
                nc.scalar.dma_start(out=xt1t[:], in_=x_in[c * DIM + 128:(c + 1) * DIM, :])
                xt1 = xt1t[0:64]

                # ---- QK projections -> per-head qf/kf tiles ----
                # qf_h rows: 0-63 q_h*scale, 64-80 rel logits, 81-87 static
                # query-mask rows, 88 = -30.  kf_h rows: 0-63 k_h, 64-88
                # static key-side pattern (pos one-hots, batch indicators,
                # ones).  One fused 89-deep matmul per (g,h) computes
                # scores + rel + mask in a single pass.
                qf = [sp.tile([89, TC], f16, tag=f"qf{h}", name=f"qf{h}")
                      for h in range(H)]
                kf = [sp.tile([89, TC], f16, tag=f"kf{h}", name=f"kf{h}")
                      for h in range(H)]
                for h in range(H):
                    nc.sync.dma_start(out=kf[h][64:89, :], in_=emlk_d[:])
                    nc.scalar.dma_start(out=qf[h][81:89, :], in_=maskq_d[:])
                for m in range(3):
                    for ni, (n0, nw) in enumerate(NT512):
                        pq = ps.tile([128, 512], f32, tag="ps32")
                        nc.tensor.matmul(pq[:, 0:nw], wqk0[:, m * 128:(m + 1) * 128],
                                         xt0[:, n0:n0 + nw], start=True, stop=False)
                        nc.tensor.matmul(pq[:, 0:nw], wqk1[:, m * 128:(m + 1) * 128],
                                         xt1[:, n0:n0 + nw], start=False, stop=True)
                        if (m * 5 + ni) % 2 == 0:
                            nc.vector.tensor_copy(qf[m][0:64, n0:n0 + nw], pq[0:64, 0:nw])
                            nc.scalar.copy(kf[m][0:64, n0:n0 + nw], pq[64:128, 0:nw])
                        else:
                            nc.scalar.copy(qf[m][0:64, n0:n0 + nw], pq[0:64, 0:nw])
                            nc.vector.tensor_copy(kf[m][0:64, n0:n0 + nw], pq[64:128, 0:nw])

                # ---- rel projections -> qf rows 64..80 ----
                xt0v = xt0[:].rearrange("p (b v) -> p b v", v=V)
                xt1v = xt1.rearrange("p (b v) -> p b v", v=V)
                for ip in range(5):          # packs of 4 positions
                    n = min(4, V - ip * 4)
                    pr = ps.tile([96, 512], f32, tag="ps32")
                    for u in range(n):
                        i = ip * 4 + u
                        nc.tensor.matmul(pr[:, u * 128:u * 128 + 128],
                                         wrel0[:, i * 96:(i + 1) * 96],
                                         xt0v[:, :, i], start=True, stop=False)
                        nc.tensor.matmul(pr[:, u * 128:u * 128 + 128],
                                         wrel1[:, i * 96:(i + 1) * 96],
                                         xt1v[:, :, i], start=False, stop=True)
                    # evict rows 32h..32h+16 per head, all n positions at once
                    prv = pr[:].rearrange("p (u b) -> p b u", b=128)
                    for h in range(H):
                        src = prv[32 * h:32 * h + 17, :, 0:n]
                        dst = qf[h][64:81, :].rearrange(
                            "p (b v) -> p b v", v=V)[:, :, ip * 4:ip * 4 + n]
                        if (ip + h) % 2 == 0:
                            nc.vector.tensor_copy(dst, src)
                        else:
                            nc.scalar.copy(dst, src)

                # ---- scores: one fused 89-deep matmul per (g,h), exp ----
                attn = sp.tile([119, 57 * 128], f16, tag="attn")
                for pk in range(15):         # packs of 4 (g,h) tiles; 57=14*4+1
                    n = min(4, 57 - pk * 4)
                    pd = ps.tile([119, 512], f32, tag="ps32")
                    for u in range(n):
                        idx = pk * 4 + u
                        g, h = divmod(idx, H)
                        t0, kn, qn = _gdims(g)
                        o = u * 128
                        nc.tensor.matmul(pd[0:kn, o:o + qn],
                                         kf[h][:, t0:t0 + kn],
                                         qf[h][:, t0:t0 + qn],
                                         start=True, stop=True)
                    nc.scalar.activation(attn[:, pk * 512:pk * 512 + n * 128],
                                         pd[:, 0:n * 128], EXP)

                # ---- V projection (overlapped 128-token groups) ----
                vt = sp.tile([119, G * 195], f16, tag="vt")
                nc.gpsimd.memset(
                    vt[:].rearrange("p (g hh c) -> p g hh c", hh=3, c=65)[:, :, :, 64:65],
                    1.0)
                for gp in range(10):         # packs of 2 groups
                    n = min(2, G - gp * 2)
                    pv = ps.tile([119, 384], f32, tag="ps32")
                    for u in range(n):
                        g = gp * 2 + u
                        t0, kn, qn = _gdims(g)
                        gk = slice(t0, t0 + kn)
                        nc.tensor.matmul(pv[0:kn, u * 192:u * 192 + 192],
                                         xt0[:, gk], wv0[:], start=True, stop=False)
                        nc.tensor.matmul(pv[0:kn, u * 192:u * 192 + 192],
                                         xt1[:, gk], wv1[:], start=False, stop=True)
                    g0 = gp * 2
                    src = pv[:].rearrange("p (g hh c) -> p g hh c", hh=3, c=64)[:, 0:n]
                    dst = vt[:].rearrange("p (g hh c) -> p g hh c", hh=3, c=65)[
                        :, g0:g0 + n, :, 0:64]
                    if gp % 2 == 0:
                        nc.vector.tensor_copy(dst, src)
                    else:
                        nc.scalar.copy(dst, src)

                # ---- attention @ V (+denominator), normalize via TT ----
                avout = sp.tile([119, G * 192], f16, tag="avout")
                recip = sp.tile([119, G * H], f32, tag="recip")
                vtv = vt[:].rearrange("p (g c) -> p g c", c=195)
                for gp in range(10):         # packs of 2 groups
                    n = min(2, G - gp * 2)
                    pa = ps.tile([119, 390], f32, tag="ps32")
                    for u in range(n):
                        g = gp * 2 + u
                        t0, kn, qn = _gdims(g)
                        for h in range(H):
                            idx = g * H + h
                            nc.tensor.matmul(
                                pa[0:qn, u * 195 + 65 * h:u * 195 + 65 * h + 65],
                                attn[0:kn, idx * 128:idx * 128 + qn],
                                vtv[0:kn, g, 65 * h:65 * h + 65],
                                start=True, stop=True)
                    g0 = gp * 2
                    pav = pa[:].rearrange("p (g hh c) -> p g hh c", hh=3, c=65)
                    nc.vector.reciprocal(
                        recip[:, g0 * H:(g0 + n) * H].rearrange(
                            "p (g hh) -> p g hh", hh=3),
                        pav[:, 0:n, :, 64])
                    rb = recip[:, g0 * H:(g0 + n) * H].rearrange(
                        "p (g hh) -> p g hh", hh=3).unsqueeze(3).broadcast_to(
                        (119, n, 3, 64))
                    nc.vector.tensor_mul(
                        avout[:].rearrange("p (g hh c) -> p g hh c", hh=3, c=64)[
                            :, g0:g0 + n],
                        pav[:, 0:n, :, 0:64], rb)

                # ---- transpose avout to feature-major ----
                aot0 = sp.tile([128, G * 128], f16, tag="aot0")
                aot1 = sp.tile([64, G * 128], f16, tag="aot1")
                for gp in range(5):          # packs of 4 groups
                    n = min(4, G - gp * 4)
                    pta = pst.tile([128, 512], f16, tag="pst")
                    ptb = pst.tile([64, 512], f16, tag="pstb")
                    for u in range(n):
                        g = gp * 4 + u
                        t0, kn, qn = _gdims(g)
                        nc.tensor.transpose(pta[:, u * 128:u * 128 + qn],
                                            avout[0:qn, g * 192:g * 192 + 128],
                                            ident[0:qn, 0:qn])
                        nc.tensor.transpose(ptb[:, u * 128:u * 128 + qn],
                                            avout[0:qn, g * 192 + 128:g * 192 + 192],
                                            ident[0:qn, 0:qn])
                    cs = slice(gp * 512, gp * 512 + n * 128)
                    nc.vector.tensor_copy(aot0[:, cs], pta[:, 0:n * 128])
                    nc.scalar.copy(aot1[:, cs], ptb[:, 0:n * 128])

                # ---- output projection ----
                fin = sp.tile([119, G * 192], f16, tag="fin")
                for gp in range(10):
                    n = min(2, G - gp * 2)
                    po = ps.tile([119, 384], f32, tag="ps32")
                    for u in range(n):
                        g = gp * 2 + u
                        t0, kn, qn = _gdims(g)
                        nc.tensor.matmul(po[0:qn, u * 192:u * 192 + 192],
                                         aot0[:, g * 128:g * 128 + qn],
                                         wout0[:], start=True, stop=False)
                        nc.tensor.matmul(po[0:qn, u * 192:u * 192 + 192],
                                         aot1[:, g * 128:g * 128 + qn],
                                         wout1[:], start=False, stop=True)
                    g0 = gp * 2
                    dst = fin[:, g0 * 192:(g0 + n) * 192]
                    if gp % 2 == 0:
                        nc.scalar.copy(dst, po[:, 0:n * 192])
                    else:
                        nc.vector.tensor_copy(dst, po[:, 0:n * 192])

                # ---- store ----
                nc.sync.dma_start(
                    out=y_out[r0:r0 + 18 * GS, :].rearrange("(g p) d -> p g d", p=GS),
                    in_=fin[:].rearrange("p (g d) -> p g d", d=192)[:, 0:18, :])
                nc.sync.dma_start(
                    out=y_out[r0 + 18 * GS:r0 + TC, :],
                    in_=fin[0:34, 18 * 192:19 * 192])

    nc.finalize()
    return nc


def kernel(x, W_qkv, b_qkv, key_rel, key_rel_diag, W_out, b_out):
    from concourse.bass_utils import run_bass_kernel_spmd

    # pre-transpose per chunk on host: (core, chunk, TC, DIM) -> (core, chunk, DIM, TC)
    xt = np.ascontiguousarray(
        np.asarray(x, dtype=np.float16).reshape(NCORES, NCHUNK, TC, DIM)
        .transpose(0, 1, 3, 2)).reshape(NCORES, NCHUNK * DIM, TC)
    consts = _build_host_constants(
        np.asarray(W_qkv, np.float32), np.asarray(b_qkv, np.float32),
        np.asarray(key_rel, np.float32), np.asarray(key_rel_diag, np.float32),
        np.asarray(W_out, np.float32), np.asarray(b_out, np.float32))

    if "nc" not in _CACHED:
        _CACHED["nc"] = _build_bass()
    nc = _CACHED["nc"]

    in_maps = [dict(consts, x=xt[k]) for k in range(NCORES)]
    res = run_bass_kernel_spmd(nc, in_maps, core_ids=list(range(NCORES)))
    _CACHED["last_result"] = res
    out = np.stack([res.results[k]["y"] for k in range(NCORES)], axis=0)
    return out.reshape(B, V, DIM).astype(np.float32)
